# revision 20
# baseline (speedup 1.0000x reference)
"""Trainium2 Bass kernel for a 3rd-order HONU layer.

Math: out[b] = sum_{i<=j<=k} w3[i,j,k] * xb[b,i] * xb[b,j] * xb[b,k]
with xb = [1, x] (129 features), w3 = `weight` in lexicographic
combinations_with_replacement order (366145 entries).

Restructuring (no gathers on device):
  - Pairs (j,k), j<=k, lex order; pair index (j,k) -> Q(j) + (k-j).
  - Dense W2[129, 8385]: W2[i, p(j,k)] = w3[i,j,k] for i<=j else 0;
    contiguous block-copy from the lexicographic weight layout.
  - out[b] = sum_p (xb[b,j]*xb[b,k]) * U[b,p],  U = xb @ W2.

Sharding (combination axis across 8 cores, SPMD-uniform program):
  - j round-robin: core c, slot s in [0,17) handles j = 8s+c; slot width
    fixed at 129-8s (tail zero-padded) so the program is identical on all
    cores; per-core differences live only in the data.
  - The single i=128 weight (pair (128,128), w3[128,128,128]*x127^3) is
    added on the host, so the device contraction is exactly K=128.
  - xsh[b,t] = xb[b,t+c] (host-shifted xb) lets the device form monomial
    pairs with compile-time offsets: pair (j=8s+c, k=j+u) has
    P = xsh[:,8s] * xsh[:,8s+u].

Engine split (measured rates: vector ~1.05ns/col + ~240ns/op,
scalar ~330ns/op + ~1.5ns/col, gpsimd ~250ns/op + ~12.5ns/col):
  - Sync queue (HWDGE): weight chunks 0-2 (bf16 by default; set
    HONU_MM_DT=float8e4 for fp8 weights pre-scaled x64 on the host,
    un-scaled in the host combine). xbt rides in the same tensor.
  - Scalar queue (HWDGE): one fp32 "xall" = [xsh half0 | xsh half1].
  - Scalar engine: primes the ACT table with a dummy op, then prebuilds
    P (pair products) for slots 7-12 of half0 and 7-11 of half1.
  - GpSimd: prebuilds P for slots 13-16 of half0 and 12-16 of half1.
  - PE: 6 matmuls U = xbT.T @ W2slice (slot groups 0-2 / 3-6 / 7-16
    per half, widths 363/372/370, own PSUM bank each).
  - Vector: slots 0-6 as fused scalar_tensor_tensor per slot (14 ops),
    slots 7-16 as one wide STT per half over the prebuilt P; each op's
    row-sum lands in its own acc column (16 cols; the discarded product
    tensor goes to a bf16 scratch), then a same-engine pipeline drain
    flushes the DVE accumulator writebacks.
  - Sync DMAs acc [128,16] straight out; the host transposes/sums.
Each weight-chunk DMA gets its own semaphore: a DMA fans out over the
16 DMA engines which each inc the sem by 1, so increments from
different DMAs on one queue interleave and a shared counter would not
order chunk0 against chunk1.
"""

import os

import numpy as np

import concourse.bass as bass
import concourse.mybir as mybir
from concourse.bass_utils import run_bass_kernel_spmd

# ---- problem constants (hardcoded; kernel.py must be self-contained) ----
N = 129                      # features incl. bias column
B = 256                      # batch
N_CORES = 8
NPAIR = N * (N + 1) // 2     # 8385
N_SLOTS = 17
SLOT_W = [N - 8 * s for s in range(N_SLOTS)]           # 129, 121, ..., 1
SLOT_OFF = [0]
for _w in SLOT_W:
    SLOT_OFF.append(SLOT_OFF[-1] + _w)
L = SLOT_OFF[-1]             # 1105 local columns per core
# PSUM tile groups of whole slots; widths 363, 372, 370 (all <= 512)
GROUPS = [(0, 3), (3, 7), (7, 17)]
G2_OFF = SLOT_OFF[7]         # 735; group-2 columns are 735..1105
G2_W = L - G2_OFF            # 370
XC_H = [0, N]                # xall xsh start col per half
XALL_W = 2 * N               # 258

# P prebuild split: (engine, half, slot range)
SCALAR_P = [(0, range(7, 13)), (1, range(7, 12))]
GPSIMD_P = [(0, range(13, 17)), (1, range(12, 17))]

# vector op order: (half, kind) where kind is a slot index or 'wide';
# acc column = position in this list
VOPS = ([(0, s) for s in range(0, 3)] + [(1, s) for s in range(0, 3)]
        + [(0, s) for s in range(3, 7)] + [(1, s) for s in range(3, 7)]
        + [(0, "wide"), (1, "wide")])
HALF_OF_COL = [h for h, _ in VOPS]
NCOL = len(VOPS)             # 16

_MM_DT_NAME = os.environ.get("HONU_MM_DT", "bfloat16")
_MM_DT = getattr(mybir.dt, _MM_DT_NAME)
_F32 = mybir.dt.float32
W_SCALE = 64.0 if _MM_DT_NAME.startswith("float8") else 1.0

LAST_RESULTS = None          # BassKernelResults of the most recent run


def _np_mm_dtype():
    import ml_dtypes
    if _MM_DT_NAME == "bfloat16":
        return ml_dtypes.bfloat16
    if _MM_DT_NAME == "float8e4":
        return ml_dtypes.float8_e4m3fn
    return np.float32


def _build_bass():
    nc = bass.Bass()
    mmw_d = nc.dram_tensor("mmw", [128, B + L], _MM_DT, kind="ExternalInput")
    xall_d = nc.dram_tensor("xall", [128, XALL_W], _F32, kind="ExternalInput")
    out_d = nc.dram_tensor("out", [128, NCOL], _F32, kind="ExternalOutput")

    mult = mybir.AluOpType.mult

    c0 = B + SLOT_OFF[GROUPS[0][1]]     # end col of weight chunk 0
    c1 = B + SLOT_OFF[GROUPS[1][1]]     # end col of weight chunk 1

    from contextlib import ExitStack
    with ExitStack() as ctx:
        ec = ctx.enter_context
        mmw_t = ec(nc.sbuf_tensor("mmw_t", [128, B + L], _MM_DT))
        xall_t = ec(nc.sbuf_tensor("xall_t", [128, XALL_W], _F32))
        p0_t = ec(nc.sbuf_tensor("p0_t", [128, G2_W], _F32))
        p1_t = ec(nc.sbuf_tensor("p1_t", [128, G2_W], _F32))
        scr_t = ec(nc.sbuf_tensor("scr_t", [128, 768], mybir.dt.bfloat16))
        acc_t = ec(nc.sbuf_tensor("acc_t", [128, NCOL], _F32))
        dead_t = ec(nc.sbuf_tensor("dead_t", [128, 1], _F32))
        psums = [ec(nc.psum_tensor(f"ps{i}", [128, 512], _F32))
                 for i in range(6)]
        wS0 = ec(nc.semaphore("wS0"))    # weight chunk 0 complete
        wS1 = ec(nc.semaphore("wS1"))    # weight chunk 1 complete
        wS2 = ec(nc.semaphore("wS2"))    # weight chunk 2 complete
        wA = ec(nc.semaphore("wA"))      # xall complete
        p0_sem = ec(nc.semaphore("p0_sem"))   # P half0 ready (2 producers)
        p1_sem = ec(nc.semaphore("p1_sem"))   # P half1 ready (2 producers)
        pe_sem = ec(nc.semaphore("pe_sem"))
        v_sem = ec(nc.semaphore("v_sem"))
        block = ec(nc.Block())
        p_ts = [p0_t, p1_t]
        p_sems = [p0_sem, p1_sem]

        def build_p(eng_iface, plan):
            for h, slots in plan:
                xc = XC_H[h]
                ins = None
                for s in slots:
                    w = SLOT_W[s]
                    lo = SLOT_OFF[s] - G2_OFF
                    if eng_iface is nc.scalar:
                        ins = nc.scalar.mul(
                            p_ts[h][:, lo:lo + w],
                            xall_t[:, xc + 8 * s:xc + 8 * s + w],
                            xall_t[:, xc + 8 * s:xc + 8 * s + 1],
                        )
                    else:
                        ins = nc.gpsimd.tensor_scalar_mul(
                            p_ts[h][:, lo:lo + w],
                            xall_t[:, xc + 8 * s:xc + 8 * s + w],
                            xall_t[:, xc + 8 * s:xc + 8 * s + 1],
                        )
                ins.then_inc(p_sems[h], 1)

        @block.sync
        def _(sync):
            sync.dma_start(mmw_t[:, 0:c0], mmw_d[:, 0:c0]).then_inc(wS0, 16)
            sync.dma_start(mmw_t[:, c0:c1], mmw_d[:, c0:c1]).then_inc(wS1, 16)
            sync.dma_start(mmw_t[:, c1:B + L],
                           mmw_d[:, c1:B + L]).then_inc(wS2, 16)
            sync.wait_ge(v_sem, NCOL + 1)
            sync.dma_start(out_d[:, :], acc_t[:, :]).then_inc(v_sem, 16)
            sync.wait_ge(v_sem, NCOL + 17)

        @block.scalar
        def _(scalar):
            # prime the ACT table (1.3us one-time load) on garbage data
            # before any real dependency
            nc.scalar.mul(dead_t[:, 0:1], dead_t[:, 0:1], 1.0)
            scalar.dma_start(xall_t[:], xall_d[:]).then_inc(wA, 16)
            scalar.wait_ge(wA, 16)
            build_p(nc.scalar, SCALAR_P)

        @block.gpsimd
        def _(gpsimd):
            gpsimd.wait_ge(wA, 16)
            build_p(nc.gpsimd, GPSIMD_P)

        @block.tensor
        def _(tensor):
            for gi, (s0, s1) in enumerate(GROUPS):
                g0c, g1c = SLOT_OFF[s0], SLOT_OFF[s1]
                tensor.wait_ge([wS0, wS1, wS2][gi], 16)
                for h in range(2):
                    nc.tensor.matmul(
                        psums[2 * gi + h][:, :g1c - g0c],
                        lhsT=mmw_t[:, h * 128:(h + 1) * 128],
                        rhs=mmw_t[:, B + g0c:B + g1c],
                        start=True, stop=True,
                    ).then_inc(pe_sem, 1)

        @block.vector
        def _(vector):
            vector.wait_ge(wA, 16)
            prev_block = None
            for col, (h, kind) in enumerate(VOPS):
                xc = XC_H[h]
                if kind == "wide":
                    vector.wait_ge(pe_sem, 5 + h)
                    vector.wait_ge(p_sems[h], 2)
                    nc.vector.scalar_tensor_tensor(
                        out=scr_t[:, 384 * h:384 * h + G2_W],
                        in0=p_ts[h][:, :],
                        scalar=1.0,
                        in1=psums[4 + h][:, :G2_W],
                        op0=mult, op1=mult,
                        accum_out=acc_t[:, col:col + 1],
                    ).then_inc(v_sem, 1)
                else:
                    s = kind
                    gi = 0 if s < 3 else 1
                    g0c = SLOT_OFF[GROUPS[gi][0]]
                    w = SLOT_W[s]
                    lo = SLOT_OFF[s]
                    if prev_block != (gi, h):
                        vector.wait_ge(pe_sem, 2 * gi + h + 1)
                        prev_block = (gi, h)
                    nc.vector.scalar_tensor_tensor(
                        out=scr_t[:, lo - g0c:lo - g0c + w],
                        in0=xall_t[:, xc + 8 * s:xc + 8 * s + w],
                        scalar=xall_t[:, xc + 8 * s:xc + 8 * s + 1],
                        in1=psums[2 * gi + h][:, lo - g0c:lo - g0c + w],
                        op0=mult, op1=mult,
                        accum_out=acc_t[:, col:col + 1],
                    ).then_inc(v_sem, 1)
            # same-engine pipeline drain flushes the DVE accumulator
            # writebacks before the out DMA reads acc
            vector.drain().then_inc(v_sem, 1)
    return nc


_NC_CACHE = None


def _get_nc():
    global _NC_CACHE
    if _NC_CACHE is None:
        _NC_CACHE = _build_bass()
    return _NC_CACHE


def _host_prep(x, weight):
    """Build per-core input maps from the full inputs."""
    mmdt = _np_mm_dtype()
    xb = np.concatenate([np.ones((B, 1), np.float32), x], axis=1)  # [256,129]

    # Global dense W2 [129, 8385] (rows i=0..127 used on device)
    W2 = np.zeros((N, NPAIR), np.float32)
    off = 0
    for i in range(N):
        m = (N - i) * (N - i + 1) // 2
        W2[i, NPAIR - m:] = weight[off:off + m]
        off += m

    def Q(j):
        return j * N - j * (j - 1) // 2

    xbt = np.ascontiguousarray(xb[:, :128].T)                    # [128, 256]

    in_maps = []
    for c in range(N_CORES):
        W2L = np.zeros((128, L), np.float32)
        for s in range(N_SLOTS):
            j = 8 * s + c
            if j >= N:
                continue
            w = N - j
            W2L[:, SLOT_OFF[s]:SLOT_OFF[s] + w] = W2[:128, Q(j):Q(j) + w]
        xsh = np.zeros((B, N), np.float32)
        xsh[:, :N - c] = xb[:, c:]
        mmw = np.concatenate([xbt, W2L * W_SCALE], axis=1).astype(mmdt)
        xall = np.concatenate([xsh[0:128, :], xsh[128:256, :]], axis=1)
        in_maps.append({
            "mmw": np.ascontiguousarray(mmw),
            "xall": np.ascontiguousarray(xall),
        })
    return in_maps


def kernel(x, weight, comb_idx=None):
    """Full inputs in, full output out. comb_idx is implied by the fixed
    lexicographic layout and is not used."""
    global LAST_RESULTS
    x = np.asarray(x, dtype=np.float32)
    weight = np.asarray(weight, dtype=np.float32)
    in_maps = _host_prep(x, weight)
    nc = _get_nc()
    res = run_bass_kernel_spmd(nc, in_maps, list(range(N_CORES)))
    LAST_RESULTS = res
    out = np.zeros((B,), np.float64)
    for r in res.results:
        o = r["out"].astype(np.float64)          # [128, NCOL]
        for col, h in enumerate(HALF_OF_COL):
            out[h * 128:(h + 1) * 128] += o[:, col]
    out /= W_SCALE
    # the single i=128 term (pair (128,128), i=j=k=128), kept off-device
    # so the device contraction is exactly K=128
    out += weight[-1].astype(np.float64) * x[:, 127].astype(np.float64) ** 3
    return out.astype(np.float32)[:, None]
